# revision 24
# baseline (speedup 1.0000x reference)
"""Trainium2 Bass kernel for AttentionSocialPooling.

Strategy (8 cores, data parallel over batch B=8; core m handles batch b=m):
For each (b,t) the N x N pairwise attention MLP is decomposed as
  hidden[i,j,a] = relu(u[i,a] + v[j,a]),  u = pos@(W1p-W1d)+b1, v = pos@W1d
(all scaled by |W2[a]|, channels permuted so positive-W2 channels come first).
H[j, (i,a)] is materialized by one PE matmul: lhsT rows = [v_hi; v_lo; 1; 1]
(bf16 hi/lo split for ~fp32 accuracy), rhs rows = [delta; delta; u_hi; u_lo]
where delta is a constant block-identity pattern.  relu on ACT engine; the
signed channel reduction on DVE (two strided reduces); dist^2 via a second
fp32 matmul (|pi|^2 + |pj|^2 - 2 pi.pj) with a -3000*I rank-128 matmul to
robustly exclude the diagonal; mask = ((d2-2500)*d2 < 0); att = sigmoid;
final row sums via PE matmuls with w^T / mask^T as the stationary operand.
"""

import numpy as np
import ml_dtypes

B, T, N, C, A = 8, 64, 128, 2, 16
R2 = 2500.0
BIG = 3000.0

bf16 = ml_dtypes.bfloat16

_CACHE = {}


def _host_prep(positions, W1, b1, W2, b2):
    pos = np.asarray(positions, dtype=np.float32)
    W1 = np.asarray(W1, dtype=np.float32)
    b1 = np.asarray(b1, dtype=np.float32)
    W2 = np.asarray(W2, dtype=np.float32)
    b2 = np.asarray(b2, dtype=np.float32)

    W1p, W1d = W1[:C], W1[C:]
    w2 = W2[:, 0]
    pos_idx = np.where(w2 >= 0)[0]
    neg_idx = np.where(w2 < 0)[0]
    npos, nneg = len(pos_idx), len(neg_idx)
    np2 = npos + (npos % 2)
    nn2 = nneg + (nneg % 2)
    A2 = np2 + nn2

    # permuted + |W2|-scaled channel coefficient matrices (zero cols = pads)
    Wu2 = np.zeros((C, A2), np.float32)
    Wd2 = np.zeros((C, A2), np.float32)
    b1v = np.zeros((A2,), np.float32)
    for k, a in enumerate(pos_idx):
        g = abs(w2[a])
        Wu2[:, k] = g * (W1p[:, a] - W1d[:, a])
        Wd2[:, k] = g * W1d[:, a]
        b1v[k] = g * b1[a]
    for k, a in enumerate(neg_idx):
        g = abs(w2[a])
        Wu2[:, np2 + k] = g * (W1p[:, a] - W1d[:, a])
        Wd2[:, np2 + k] = g * W1d[:, a]
        b1v[np2 + k] = g * b1[a]

    u = pos @ Wu2 + b1v          # [B,T,N,A2]
    v = pos @ Wd2                # [B,T,N,A2]

    uhi = u.astype(bf16)
    ulo = (u - uhi.astype(np.float32)).astype(bf16)
    vhi = v.astype(bf16)
    vlo = (v - vhi.astype(np.float32)).astype(bf16)

    # lhsT for the H matmul: rows [1; 1; v_hi(A2); v_lo(A2)] (per core)
    vT = np.empty((B, 2 * A2 + 2, T * N), dtype=bf16)
    vT[:, 0:2] = np.asarray(1.0, dtype=bf16)
    vT[:, 2:A2 + 2] = vhi.transpose(0, 3, 1, 2).reshape(B, A2, T * N)
    vT[:, A2 + 2:] = vlo.transpose(0, 3, 1, 2).reshape(B, A2, T * N)

    # per-t rhs rows for u: [T, 2, N*A2] bf16 (per core)
    uflat = np.empty((B, T, 2, N * A2), dtype=bf16)
    uflat[:, :, 0] = uhi.reshape(B, T, N * A2)
    uflat[:, :, 1] = ulo.reshape(B, T, N * A2)

    # block-identity delta pattern, stacked twice (hi+lo rows) [2*A2, N*A2]
    delta1 = np.zeros((A2, N * A2), dtype=bf16)
    for a in range(A2):
        delta1[a, a::A2] = np.asarray(1.0, dtype=bf16)
    delta = np.concatenate([delta1, delta1], axis=0)

    # dist^2 matmul operands, fp16 hi/lo split (K=10, cross terms kept)
    f16 = np.float16
    pos64 = pos.astype(np.float64)
    n2 = (pos64 ** 2).sum(-1)        # [B,T,N] (float64)
    px = pos64[..., 0].reshape(B, T * N)
    py = pos64[..., 1].reshape(B, T * N)
    n2f = n2.reshape(B, T * N)

    def hilo(x):
        hi = x.astype(f16)
        lo = (x - hi.astype(np.float64)).astype(f16)
        return hi, lo

    pxh, pxl = hilo(px)
    pyh, pyl = hilo(py)
    n2h, n2l = hilo(n2f)
    m2pxh, m2pxl = hilo(-2 * px)
    m2pyh, m2pyl = hilo(-2 * py)
    ones = np.ones_like(pxh)
    zeros = np.zeros_like(pxh)
    lhsTd = np.stack([pxh, pxh, pxl, pyh, pyh, pyl, ones, ones, n2h, n2l],
                     axis=1).astype(f16)                     # [B,10,T*N]
    rhsd = np.stack([m2pxh, m2pxl, m2pxh, m2pyh, m2pyl, m2pyh, n2h, n2l,
                     ones, ones], axis=1).astype(f16)

    # final-matmul rhs, fp16 hi/lo: per t 6 cols (pxh,pyh,1, pxl,pyl,0)
    pos3 = np.empty((B, N, T * 6), f16)
    p6 = pos3.reshape(B, N, T, 6)
    p6[..., 0] = pxh.reshape(B, T, N).transpose(0, 2, 1)
    p6[..., 1] = pyh.reshape(B, T, N).transpose(0, 2, 1)
    p6[..., 2] = 1.0
    p6[..., 3] = pxl.reshape(B, T, N).transpose(0, 2, 1)
    p6[..., 4] = pyl.reshape(B, T, N).transpose(0, 2, 1)
    p6[..., 5] = 0.0

    negI = (-BIG * np.eye(N)).astype(bf16)
    eye = np.eye(N, dtype=bf16)

    return dict(vT=vT, uflat=uflat, delta=delta, lhsTd=lhsTd, rhsd=rhsd,
                pos3=pos3, negI=negI, eye=eye, A2=A2, np2=np2, nn2=nn2,
                b2=float(b2[0]))


def _build_program(A2, np2, nn2, b2val):
    import concourse.bacc as bacc
    import concourse.mybir as mybir
    import concourse.tile as tile

    f32 = mybir.dt.float32
    f32r = mybir.dt.float32r
    f16 = mybir.dt.float16
    bfl = mybir.dt.bfloat16
    Alu = mybir.AluOpType
    Act = mybir.ActivationFunctionType
    X = mybir.AxisListType.X

    K2 = 2 * A2 + 2
    NA = N * A2
    HALF = 64 * A2          # columns per PSUM half-tile

    nc = bacc.Bacc()

    vT_p = nc.declare_dram_parameter("vT", [K2, T * N], bfl, isOutput=False)
    uflat_p = nc.declare_dram_parameter("uflat", [T, 2, NA], bfl, isOutput=False)
    lhsTd_p = nc.declare_dram_parameter("lhsTd", [10, T * N], f16, isOutput=False)
    rhsd_p = nc.declare_dram_parameter("rhsd", [10, T * N], f16, isOutput=False)
    delta_p = nc.declare_dram_parameter("delta", [2 * A2, NA], bfl, isOutput=False)
    pos3_p = nc.declare_dram_parameter("pos3", [N, T * 6], f16, isOutput=False)
    negI_p = nc.declare_dram_parameter("negI", [N, N], bfl, isOutput=False)
    eye_p = nc.declare_dram_parameter("eye", [N, N], bfl, isOutput=False)
    out_p = nc.declare_dram_parameter("out", [T, N, C], f32, isOutput=True)

    hbufs = 3 if A2 <= 16 else 2

    with tile.TileContext(nc) as tc:
        with (
            tc.tile_pool(name="pers", bufs=1) as pers,
            tc.tile_pool(name="hpsum", bufs=hbufs, space="PSUM") as hpsum,
            tc.tile_pool(name="dpsum", bufs=1, space="PSUM") as dpsum,
            tc.tile_pool(name="fpsum", bufs=1, space="PSUM") as fpsum,
            tc.tile_pool(name="work", bufs=2) as work,
        ):
            vT_s = pers.tile([K2, T * N], bfl, tag="vT")
            lhsTd_s = pers.tile([10, T * N], f16, tag="lhsTd")
            rhsd_s = pers.tile([10, T * N], f16, tag="rhsd")
            pos3_s = pers.tile([N, T * 6], f16, tag="pos3")
            negI_s = pers.tile([N, N], bfl, tag="negI")
            eye_s = pers.tile([N, N], bfl, tag="eye")
            rhsH0 = pers.tile([K2, NA], bfl, tag="rhsH0")
            rhsH1 = pers.tile([K2, NA], bfl, tag="rhsH1")
            rhsH = [rhsH0, rhsH1]

            nc.gpsimd.dma_start(vT_s[:], vT_p[:])
            nc.gpsimd.dma_start(lhsTd_s[:], lhsTd_p[:])
            nc.gpsimd.dma_start(rhsd_s[:], rhsd_p[:])
            nc.gpsimd.dma_start(pos3_s[:], pos3_p[:])
            nc.gpsimd.dma_start(negI_s[:], negI_p[:])
            nc.gpsimd.dma_start(eye_s[:], eye_p[:])
            for i in range(2):
                nc.gpsimd.dma_start(rhsH[i][2:2 * A2 + 2, :], delta_p[:])

            # matmul column chunking within a PSUM half (bank-aligned, <=512)
            chunks = []
            off = 0
            while off < HALF:
                cn = min(512, HALF - off)
                chunks.append((off, cn))
                off += cn

            pf = None
            for t in range(T):
                g = t % 8
                if g == 0:
                    pf = fpsum.tile([N, 64], f32, tag="F")
                rh = rhsH[t % 2]
                nc.sync.dma_start(rh[0:2, :], uflat_p[t])

                Rt = work.tile([N, NA], f16, tag="R")
                for h in range(2):
                    ph = hpsum.tile([N, HALF], f32, tag="H")
                    for (off, cn) in chunks:
                        nc.tensor.matmul(
                            ph[:, off:off + cn],
                            vT_s[:, t * N:(t + 1) * N],
                            rh[:, h * HALF + off:h * HALF + off + cn],
                            start=True, stop=True,
                        )
                    nc.scalar.activation(Rt[:, h * HALF:(h + 1) * HALF], ph[:], Act.Relu)

                R3 = Rt[:].rearrange("p (i a) -> p i a", a=A2)
                attP = work.tile([N, N], f32, tag="attP")
                attM = work.tile([N, N], f32, tag="attM")
                pre = work.tile([N, N], f32, tag="pre")
                if np2 and nn2:
                    nc.vector.tensor_reduce(attP[:], R3[:, :, 0:np2], axis=X, op=Alu.add)
                    nc.vector.tensor_reduce(attM[:], R3[:, :, np2:A2], axis=X, op=Alu.add)
                    nc.vector.tensor_sub(pre[:], attP[:], attM[:])
                elif np2:
                    nc.vector.tensor_reduce(pre[:], R3[:, :, 0:np2], axis=X, op=Alu.add)
                else:
                    nc.vector.tensor_reduce(attP[:], R3[:, :, np2:A2], axis=X, op=Alu.add)
                    nc.vector.tensor_scalar_mul(pre[:], attP[:], -1.0)

                pd = dpsum.tile([N, N], f32, tag="D")
                nc.tensor.matmul(pd[:], lhsTd_s[:, t * N:(t + 1) * N],
                                 rhsd_s[:, t * N:(t + 1) * N], start=True, stop=False)
                nc.tensor.matmul(pd[:], negI_s[:], eye_s[:],
                                 start=False, stop=True)

                m1 = work.tile([N, N], f16, tag="m1")
                m2 = work.tile([N, N], f16, tag="m2")
                nc.vector.tensor_scalar(m1[:], pd[:], 0.0, None, op0=Alu.is_gt)
                nc.vector.tensor_scalar(m2[:], pd[:], R2, None, op0=Alu.is_lt)
                maskt = work.tile([N, N], f16, tag="mask")
                nc.gpsimd.tensor_mul(maskt[:], m1[:], m2[:])

                attt = work.tile([N, N], f16, tag="att")
                nc.scalar.activation(attt[:], pre[:], Act.Sigmoid, bias=b2val, scale=1.0)
                wt = work.tile([N, N], f16, tag="w")
                nc.gpsimd.tensor_mul(wt[:], attt[:], maskt[:])

                nc.tensor.matmul(pf[:, 8 * g:8 * g + 6], wt[:],
                                 pos3_s[:, 6 * t:6 * t + 6], start=True, stop=True)
                nc.tensor.matmul(pf[:, 8 * g + 6:8 * g + 7], maskt[:],
                                 pos3_s[:, 6 * t + 2:6 * t + 3], start=True, stop=True)

                if g == 7:
                    # out[i,c] = (sum_j w*pos_j[c] - pos_i[c]*sum_j w) / cnt
                    # F cols per group: 0,1=num_hi 2=sum_w 3,4=num_lo 5=0 6=cnt
                    pf3 = pf[:].rearrange("p (g c) -> p g c", c=8)
                    p3 = pos3_s[:, 6 * (t - 7):6 * (t + 1)].rearrange(
                        "p (g c) -> p g c", c=6)
                    cnt8 = work.tile([N, 8], f32, tag="cnt8")
                    rcp8 = work.tile([N, 8], f32, tag="rcp8")
                    sw8 = work.tile([N, 16], f32, tag="sw8")
                    outst = work.tile([N, 16], f32, tag="outst")
                    nc.vector.tensor_scalar_max(cnt8[:], pf3[:, :, 6], 1e-6)
                    nc.vector.reciprocal(rcp8[:], cnt8[:])
                    s3 = sw8[:].rearrange("p (g c) -> p g c", c=2)
                    o3 = outst[:].rearrange("p (g c) -> p g c", c=2)
                    for c in range(2):
                        # pos_i reconstructed as hi+lo (one PSUM read per op)
                        nc.vector.tensor_add(s3[:, :, c], p3[:, :, c], p3[:, :, c + 3])
                        nc.vector.tensor_mul(s3[:, :, c], pf3[:, :, 2], s3[:, :, c])
                        nc.vector.tensor_sub(o3[:, :, c], pf3[:, :, c], s3[:, :, c])
                        nc.vector.tensor_add(o3[:, :, c], o3[:, :, c], pf3[:, :, c + 3])
                        nc.vector.tensor_mul(o3[:, :, c], o3[:, :, c], rcp8[:])
                    nc.sync.dma_start(
                        out_p[t - 7:t + 1].rearrange("t n c -> n t c"), outst[:])

    nc.compile()
    return nc


def kernel(positions, W1, b1, W2, b2, _trace=False, _trace_kwargs=None):
    from concourse.bass_utils import run_bass_kernel_spmd

    prep = _host_prep(positions, W1, b1, W2, b2)
    A2, np2, nn2, b2v = prep["A2"], prep["np2"], prep["nn2"], prep["b2"]

    key = (A2, np2, nn2, b2v)
    if key not in _CACHE:
        _CACHE[key] = _build_program(A2, np2, nn2, b2v)
    nc = _CACHE[key]

    in_maps = []
    for b in range(B):
        in_maps.append({
            "vT": np.ascontiguousarray(prep["vT"][b]),
            "uflat": np.ascontiguousarray(prep["uflat"][b]),
            "delta": prep["delta"],
            "lhsTd": np.ascontiguousarray(prep["lhsTd"][b]),
            "rhsd": np.ascontiguousarray(prep["rhsd"][b]),
            "pos3": np.ascontiguousarray(prep["pos3"][b]),
            "negI": prep["negI"],
            "eye": prep["eye"],
        })

    kw = {}
    if _trace:
        kw["trace"] = True
        if _trace_kwargs:
            kw.update(_trace_kwargs)
    res = run_bass_kernel_spmd(nc, in_maps, list(range(B)), **kw)
    out = np.stack([r["out"] for r in res.results], axis=0).astype(np.float32)
    if _trace:
        return out, res
    return out
